# revision 1
# baseline (speedup 1.0000x reference)
"""DGCNN (3x DynamicEdgeConv + MLP head) Trainium2 Bass kernel.

Data-parallel over the batch axis: 8 graphs -> 8 NeuronCores, weights
replicated. Per graph (N=2048 nodes, K=16 neighbors):
  - kNN scores s'[i,j] = 2<x_i,x_j> - |x_j|^2 via exact fp32 PE matmuls
    (ones/negsq row folded into the contraction for L1/L2)
  - top-16 via index-packed scores: the low 11 mantissa bits of each
    score are replaced by the column index (one DVE pass), then
    max8 / match_replace / max8 yield values whose low bits ARE the
    neighbor indices (no max_index passes, no duplicate hazards)
  - neighbor gather with gpsimd ap_gather; each 16-partition Q7 core
    processes its own tile's index list, so one gather instruction
    serves 8 tiles (L1: scalar features replicated on all partitions),
    4 tiles (L2: bf16-pair packed 32-row features, 4 replicas) or
    2 tiles (L3: bf16-pair packed 64-row features, 2 replicas)
  - edge MLP first linear on PE from gathered x_j (bf16) plus broadcast
    x_i term (fp32r), relu on ACT, second linear fp32r, max-pool over k
    via DVE segmented reduce; groups software-pipelined so PE/DVE work
    overlaps the Pool-engine gathers
  - final MLP in [C, N] layout fp32r, classifier + log_softmax
"""

import numpy as np

import concourse.bacc as bacc
import concourse.mybir as mybir
from concourse import tile
from concourse import bass_utils

F32 = mybir.dt.float32
F32R = mybir.dt.float32r
BF16 = mybir.dt.bfloat16
U16 = mybir.dt.uint16
U32 = mybir.dt.uint32
I16 = mybir.dt.int16
AX = mybir.AxisListType
OP = mybir.AluOpType
ACTF = mybir.ActivationFunctionType


def _r(ap):
    """fp32 -> fp32r view: 4x PE throughput for matmuls >=256 cols wide."""
    return ap.bitcast(F32R)


B, N, K = 8, 2048, 16
NT = N // 128
NEG = -3.0e38
MASK_HI = 0xFFFFF800
MASK_LO = 0x000007FF

WEIGHT_NAMES = [
    "w1a_w", "w1a_b", "w1b_w", "w1b_b",
    "w2a_w", "w2a_b", "w2b_w", "w2b_b",
    "w3a_w", "w3a_b", "w3b_w", "w3b_b",
    "m1_w", "m1_b", "m2_w", "m2_b", "m3_w", "m3_b", "m4_w", "m4_b",
]
WEIGHT_SHAPES = {
    "w1a_w": (2, 64), "w1a_b": (64,), "w1b_w": (64, 64), "w1b_b": (64,),
    "w2a_w": (128, 128), "w2a_b": (128,), "w2b_w": (128, 128), "w2b_b": (128,),
    "w3a_w": (256, 256), "w3a_b": (256,), "w3b_w": (256, 256), "w3b_b": (256,),
    "m1_w": (448, 512), "m1_b": (512,), "m2_w": (512, 512), "m2_b": (512,),
    "m3_w": (512, 256), "m3_b": (256,), "m4_w": (256, 2), "m4_b": (2,),
}


def _chunks(n, c=128):
    return [(s, min(c, n - s)) for s in range(0, n, c)]


def build_nc(repeat_loop=False, dbg_stop=None):
    nc = bacc.Bacc("TRN2", target_bir_lowering=False, debug=False)

    x_d = nc.dram_tensor("x", [N, 1], F32, kind="ExternalInput")
    w_d = {
        n: nc.dram_tensor(n, list(WEIGHT_SHAPES[n]), F32, kind="ExternalInput")
        for n in WEIGHT_NAMES
    }
    steps_d = None
    if repeat_loop:
        steps_d = nc.dram_tensor("steps", [1, 1], mybir.dt.uint32,
                                 kind="ExternalInput")
    out_d = nc.dram_tensor("out", [N, 2], F32, kind="ExternalOutput")

    with tile.TileContext(nc) as tc:
        _emit(nc, tc, x_d, w_d, out_d, steps_d, dbg_stop)
    nc.compile()
    return nc


def _emit(nc, tc, x_d, w_d, out_d, steps_d, dbg_stop=None):
    from contextlib import ExitStack

    ctx = ExitStack()
    with ctx:
        const = ctx.enter_context(tc.tile_pool(name="const", bufs=1))
        wpool = ctx.enter_context(tc.tile_pool(name="wpool", bufs=1))
        feats = ctx.enter_context(tc.tile_pool(name="feats", bufs=1))

        # ---- constants ----
        ones1 = const.tile([1, N], F32)
        nc.vector.memset(ones1[:], 1.0)
        onesc = const.tile([128, 1], F32)
        nc.vector.memset(onesc[:], 1.0)
        iota_u = const.tile([128, N], U32)
        nc.gpsimd.iota(iota_u[:], [[1, N]], channel_multiplier=0)
        mhi = const.tile([128, 1], U32)
        nc.vector.memset(mhi[:], MASK_HI)
        mlo = const.tile([128, 1], U32)
        nc.vector.memset(mlo[:], MASK_LO)

        # ---- load weights ----
        def load2d(name, row_splits=None, cast_r=False):
            rows, cols = WEIGHT_SHAPES[name]
            ts = []
            splits = row_splits or _chunks(rows)
            for i, (s, p) in enumerate(splits):
                t = wpool.tile([p, cols], F32, tag=f"w_{name}_{i}",
                               name=f"w_{name}_{i}")
                if cast_r:
                    nc.gpsimd.dma_start(t[:].bitcast(F32R),
                                        w_d[name].ap()[s:s + p, :])
                else:
                    nc.sync.dma_start(t[:], w_d[name].ap()[s:s + p, :])
                ts.append(t)
            return ts

        def load_bias_col(name):
            (n_,) = WEIGHT_SHAPES[name]
            ts = []
            for i, (s, p) in enumerate(_chunks(n_)):
                t = wpool.tile([p, 1], F32, tag=f"b_{name}_{i}",
                               name=f"b_{name}_{i}")
                nc.sync.dma_start(t[:], w_d[name].ap()[s:s + p].unsqueeze(-1))
                ts.append(t)
            return ts

        w1a = load2d("w1a_w")[0]                 # [2, 64]
        w1b = load2d("w1b_w", cast_r=True)[0]    # [64, 64]
        w2a = load2d("w2a_w")[0]                 # [128, 128]
        w2b = load2d("w2b_w", cast_r=True)[0]    # [128, 128]
        w3a = load2d("w3a_w")                    # 2 x [128, 256]
        w3b = load2d("w3b_w", cast_r=True)       # 2 x [128, 256]
        m1 = load2d("m1_w", row_splits=[(0, 64), (64, 128), (192, 128),
                                        (320, 128)], cast_r=True)
        m2 = load2d("m2_w", cast_r=True)
        m3 = load2d("m3_w", cast_r=True)
        m4 = load2d("m4_w")
        b1a = load_bias_col("w1a_b")[0]
        b1b = load_bias_col("w1b_b")[0]
        b2a = load_bias_col("w2a_b")[0]
        b2b = load_bias_col("w2b_b")[0]
        b3a = load_bias_col("w3a_b")
        b3b = load_bias_col("w3b_b")
        bm1 = load_bias_col("m1_b")
        bm2 = load_bias_col("m2_b")
        bm3 = load_bias_col("m3_b")
        bm4 = wpool.tile([1, 2], F32)
        nc.sync.dma_start(bm4[:], w_d["m4_b"].ap().unsqueeze(0))

        # wdiff = Wx - Wd (first C_in rows minus last C_in rows of w*a)
        wd1 = wpool.tile([1, 64], F32)
        nc.sync.dma_start(wd1[:], w_d["w1a_w"].ap()[1:2, :])
        wdiff1 = wpool.tile([1, 64], F32)
        nc.vector.tensor_tensor(wdiff1[:], w1a[0:1, :], wd1[:], op=OP.subtract)
        # E_all[:, 64*tp : 64*(tp+1)] is wd1 placed on partition row 16*tp,
        # zero elsewhere: matmul(E_all-slice, xg) extracts tile tp's gathered
        # scalars (parked on core tp's partitions) as a rank-1 xj term.
        E_all = wpool.tile([128, 512], F32)
        nc.vector.memset(E_all[:], 0.0)
        for tp in range(8):
            nc.sync.dma_start(E_all[16 * tp:16 * tp + 1,
                                    64 * tp:64 * tp + 64], wd1[:])
        wd2 = wpool.tile([64, 128], F32)
        nc.sync.dma_start(wd2[:], w_d["w2a_w"].ap()[64:128, :])
        wdiff2 = wpool.tile([64, 128], F32)
        nc.vector.tensor_tensor(wdiff2[:].bitcast(F32R), w2a[0:64, :], wd2[:],
                                op=OP.subtract)
        wdiff3 = wpool.tile([128, 256], F32)
        nc.vector.tensor_tensor(wdiff3[:].bitcast(F32R), w3a[0][:], w3a[1][:],
                                op=OP.subtract)

        # L2 gather weights: wd2 bf16 halves (x1 packed pairs (c, c+32)),
        # replicated at partitions 0/32/64/96
        wd2lo_b = wpool.tile([128, 128], BF16)
        wd2hi_b = wpool.tile([128, 128], BF16)
        nc.scalar.activation(wd2lo_b[0:32, :], wd2[0:32, :], ACTF.Copy)
        nc.scalar.activation(wd2hi_b[0:32, :], wd2[32:64, :], ACTF.Copy)
        for q in range(1, 4):
            nc.sync.dma_start(wd2lo_b[32 * q:32 * q + 32, :], wd2lo_b[0:32, :])
            nc.sync.dma_start(wd2hi_b[32 * q:32 * q + 32, :], wd2hi_b[0:32, :])

        # L3 gather weights: wd3 bf16 halves replicated at partitions 0/64
        wd3hi_f = wpool.tile([64, 256], F32)
        nc.sync.dma_start(wd3hi_f[:], w_d["w3a_w"].ap()[192:256, :])
        wd3lo_b = wpool.tile([128, 256], BF16)
        wd3hi_b = wpool.tile([128, 256], BF16)
        nc.scalar.activation(wd3lo_b[0:64, :], w3a[1][0:64, :], ACTF.Copy)
        nc.scalar.activation(wd3hi_b[0:64, :], wd3hi_f[:], ACTF.Copy)
        nc.sync.dma_start(wd3lo_b[64:128, :], wd3lo_b[0:64, :])
        nc.sync.dma_start(wd3hi_b[64:128, :], wd3hi_b[0:64, :])

        # ---- feature tensors ----
        x0T = feats.tile([1, N], F32)
        nc.sync.dma_start(x0T[0:1, :], x_d.ap().rearrange("n 1 -> 1 n"))
        x0rep = feats.tile([128, N], F32)  # x scalars on every partition
        nc.sync.dma_start(x0rep[0:1, :], x0T[0:1, :])
        _p = 1
        while _p < 128:
            nc.sync.dma_start(x0rep[_p:2 * _p, :], x0rep[0:_p, :])
            _p *= 2

        x1T = feats.tile([65, N], F32)     # rows 0:64 feats, row 64 = ones
        nc.vector.memset(x1T[64:65, :], 1.0)
        x1pk = feats.tile([128, N], F32)
        x2T = feats.tile([128, N], F32)
        x2pk = feats.tile([128, N], F32)
        x3Ta = feats.tile([128, N], F32)
        x3Tb = feats.tile([128, N], F32)

        body_ctx = ExitStack()
        with body_ctx:
            scr = body_ctx.enter_context(tc.tile_pool(name="scr", bufs=2))
            pss = body_ctx.enter_context(
                tc.tile_pool(name="pss", bufs=1, space="PSUM"))
            psi = body_ctx.enter_context(
                tc.tile_pool(name="psi", bufs=2, space="PSUM"))
            pse = body_ctx.enter_context(
                tc.tile_pool(name="pse", bufs=1, space="PSUM"))

            if steps_d is not None:
                steps_sb = const.tile([1, 1], mybir.dt.uint32)
                nc.sync.dma_start(steps_sb[:], steps_d.ap())
                (_, (steps_val,)) = nc.values_load_multi_w_load_instructions(
                    steps_sb[:], min_val=1, max_val=1000000,
                    skip_runtime_bounds_check=True)
                body_ctx.enter_context(tc.For_i(0, steps_val, 1))

            def layer_prep(lname, feat_lhs, c_in, merged):
                """2x rows + xsq + negsq. merged: negsq goes into row c_in
                of the twox tile so the head needs one matmul."""
                twox = scr.tile([128, N], F32, tag="twox", bufs=1,
                                name=f"twox_{lname}")
                nc.vector.tensor_scalar_mul(twox[:c_in, :],
                                            feat_lhs[:c_in, :], 2.0)
                xsq = scr.tile([128, N], F32, tag="xsq", bufs=1,
                               name=f"xsq_{lname}")
                if merged:
                    negsq = twox[c_in:c_in + 1, :]
                else:
                    # row 0 of xsq doubles as negsq home: each chunk's
                    # negsq write lands after the PE consumed that chunk
                    negsq = xsq[0:1, :]
                for j in range(4):
                    fsl = slice(j * 512, (j + 1) * 512)
                    sq_ps = psi.tile([1, 512], F32, tag="tmp")
                    nc.vector.tensor_tensor(
                        xsq[:c_in, fsl], feat_lhs[:c_in, fsl],
                        feat_lhs[:c_in, fsl], op=OP.mult)
                    nc.tensor.matmul(
                        sq_ps[:], onesc[:c_in, :], xsq[:c_in, fsl],
                        start=True, stop=True)
                    nc.scalar.activation(negsq[:, fsl], sq_ps[:], ACTF.Copy,
                                         scale=-1.0)
                return twox, negsq

            def tile_head(t, feat_lhs, c_in, twox, negsq, merged, iu_sink):
                """s'-matmul + packed top-16 for node tile t."""
                ts0 = t * 128
                tsl = slice(ts0, ts0 + 128)
                s_ps = pss.tile([128, N], F32, tag="s_ps", name="s_ps")
                rows = c_in + 1 if merged else c_in
                for j in range(4):
                    fsl = slice(j * 512, (j + 1) * 512)
                    if merged:
                        nc.tensor.matmul(
                            s_ps[:, fsl], feat_lhs[:rows, tsl],
                            twox[:rows, fsl], start=True, stop=True)
                    else:
                        nc.tensor.matmul(
                            s_ps[:, fsl], feat_lhs[:c_in, tsl],
                            twox[:c_in, fsl], start=True, stop=False)
                        nc.tensor.matmul(
                            s_ps[:, fsl], ones1[:, tsl], negsq[:, fsl],
                            start=False, stop=True)
                pk = scr.tile([128, N], F32, tag="pk", bufs=1, name="pk")
                nc.vector.scalar_tensor_tensor(
                    pk[:].bitcast(U32), s_ps[:].bitcast(U32), mhi[:],
                    iota_u[:, :], op0=OP.bitwise_and, op1=OP.bitwise_or)
                v = scr.tile([128, 16], F32, tag="v16", name="v16")
                nc.vector.max(out=v[:, 0:8], in_=pk[:])
                nc.vector.match_replace(out=pk[:], in_to_replace=v[:, 0:8],
                                        in_values=pk[:], imm_value=NEG)
                nc.vector.max(out=v[:, 8:16], in_=pk[:])
                iu_sink(v[:].bitcast(U32))

            def extract_iu32(v_u32):
                iu32 = scr.tile([128, 16], U32, tag="iu32", name="iu32")
                nc.vector.tensor_scalar(iu32[:], v_u32, mlo[:], None,
                                        op0=OP.bitwise_and)
                return iu32

            # ---------------- layer 1 ----------------
            twox1, negsq1 = layer_prep("l1", x0T, 1, False)

            def l1_heads(g):
                iu_cat = scr.tile([128, 128], U16, tag="iu_cat",
                                  name="iu_cat")
                for tp in range(8):
                    t = 8 * g + tp

                    def sink(v_u32, tp=tp):
                        iu32 = extract_iu32(v_u32)
                        nc.vector.tensor_copy(
                            iu_cat[:, 16 * tp:16 * tp + 16], iu32[:])

                    tile_head(t, x0T, 1, twox1, negsq1, False, sink)
                idx16 = scr.tile([128, 128], U16, tag="idx16", name="idx16")
                nc.sync.dma_start_transpose(idx16[:], iu_cat[:])
                xg = scr.tile([128, N], F32, tag="xg0", name="xg0")
                nc.gpsimd.ap_gather(
                    out_ap=xg[:], in_ap=x0rep[:],
                    idxs_ap=idx16[:].bitcast(I16),
                    channels=128, num_elems=N, d=1, num_idxs=N)
                return xg

            def l1_tail(t, xg):
                tp = t % 8
                ts0 = t * 128
                tsl = slice(ts0, ts0 + 128)
                esl = slice(64 * tp, 64 * tp + 64)
                for j in range(4):
                    fsl = slice(j * 512, (j + 1) * 512)
                    nsl = slice(ts0 + j * 32, ts0 + (j + 1) * 32)
                    pre = pse.tile([128, 512], F32, tag="pre", name="pre_ps")
                    nc.tensor.matmul(pre[0:64, :], E_all[:, esl],
                                     xg[:, fsl], start=True, stop=False)
                    nc.tensor.matmul(
                        pre[0:64, :], wdiff1[:],
                        x0T[0:1, nsl].unsqueeze(-1).to_broadcast([1, 32, K]),
                        start=False, stop=True)
                    h1 = scr.tile([128, 512], F32, tag="h1", bufs=2,
                                  name="h1")
                    nc.scalar.activation(h1[0:64, :].bitcast(F32R),
                                         pre[0:64, :], ACTF.Relu,
                                         bias=b1a[:])
                    h2 = pse.tile([128, 512], F32, tag="h2", name="h2_ps")
                    nc.tensor.matmul(h2[0:64, :], _r(w1b[:]), _r(h1[0:64, :]))
                    nc.vector.tensor_reduce(
                        out=x1T[0:64, nsl].bitcast(F32R),
                        in_=h2[0:64, :].rearrange("c (n k) -> c n k", k=K),
                        axis=AX.X, op=OP.max)
                nc.vector.tensor_scalar(
                    x1T[0:64, tsl].bitcast(F32R), x1T[0:64, tsl], b1b[:],
                    None, op0=OP.add)

            prev = None
            for g in range(2):
                xg = l1_heads(g)
                if prev is not None:
                    pg, pxg = prev
                    for tp in range(8):
                        l1_tail(8 * pg + tp, pxg)
                prev = (g, xg)
            pg, pxg = prev
            for tp in range(8):
                l1_tail(8 * pg + tp, pxg)

            if dbg_stop == 1:
                nc.sync.dma_start(out_d.ap().rearrange("n c -> c n"),
                                  x1T[0:2, :])
                return

            # ---------------- layer 2 ----------------
            pk2 = x1pk[0:32, :].bitcast(BF16).rearrange("c (n t) -> c n t",
                                                        t=2)
            nc.scalar.activation(pk2[:, :, 0:1], x1T[0:32, :].unsqueeze(-1),
                                 ACTF.Copy)
            nc.scalar.activation(pk2[:, :, 1:2], x1T[32:64, :].unsqueeze(-1),
                                 ACTF.Copy)
            for q in range(1, 4):
                nc.sync.dma_start(x1pk[32 * q:32 * q + 32, :], x1pk[0:32, :])

            twox2, _ = layer_prep("l2", x1T, 64, True)

            def l2_heads(g):
                iu_cat = scr.tile([128, 128], U16, tag="iu_cat",
                                  name="iu_cat")
                for q in range(4):
                    t = 4 * g + q

                    def sink(v_u32, q=q):
                        iu32 = extract_iu32(v_u32)
                        nc.vector.tensor_copy(
                            iu_cat[:, 32 * q:32 * q + 32]
                            .rearrange("p (r k) -> p r k", r=2),
                            iu32[:].unsqueeze(1).to_broadcast([128, 2, 16]))

                    tile_head(t, x1T, 64, twox2, None, True, sink)
                idx16 = scr.tile([128, 128], U16, tag="idx16", name="idx16")
                nc.sync.dma_start_transpose(idx16[:], iu_cat[:])
                xg = scr.tile([128, N], F32, tag="xg0", name="xg0")
                nc.gpsimd.ap_gather(
                    out_ap=xg[:], in_ap=x1pk[:],
                    idxs_ap=idx16[:].bitcast(I16),
                    channels=128, num_elems=N, d=1, num_idxs=N)
                return xg

            def l2_tail(t, q, xg):
                if q == 3:
                    # PE matmul base partitions must be 0/32/64: quarter 3
                    # (base 96) is copied down and processed at base 0.
                    xg3 = scr.tile([32, N], F32, tag="xg3b", bufs=1,
                                   name="xg3b")
                    nc.sync.dma_start(xg3[:], xg[96:128, :])
                    xg, q = xg3, 0
                    qsl = slice(0, 32)
                else:
                    qsl = slice(32 * q, 32 * q + 32)
                ts0 = t * 128
                tsl = slice(ts0, ts0 + 128)
                xgv = xg[qsl, :].bitcast(BF16).rearrange("c (n t) -> c n t",
                                                         t=2)
                xglo = scr.tile([128, N], BF16, tag="xglo", bufs=1,
                                name="xglo")
                xghi = scr.tile([128, N], BF16, tag="xghi", bufs=1,
                                name="xghi")
                nc.scalar.activation(xglo[qsl, :].unsqueeze(-1),
                                     xgv[:, :, 0:1], ACTF.Copy)
                nc.scalar.activation(xghi[qsl, :].unsqueeze(-1),
                                     xgv[:, :, 1:2], ACTF.Copy)
                for j in range(4):
                    fsl = slice(j * 512, (j + 1) * 512)
                    nsl = slice(ts0 + j * 32, ts0 + (j + 1) * 32)
                    pre = pse.tile([128, 512], F32, tag="pre", name="pre_ps")
                    nc.tensor.matmul(pre[:], wd2lo_b[qsl, :], xglo[qsl, fsl],
                                     start=True, stop=False)
                    nc.tensor.matmul(pre[:], wd2hi_b[qsl, :], xghi[qsl, fsl],
                                     start=False, stop=False)
                    nc.tensor.matmul(
                        pre[:], _r(wdiff2[:]),
                        _r(x1T[0:64, nsl].unsqueeze(-1)
                           .to_broadcast([64, 32, K])),
                        start=False, stop=True)
                    h1 = scr.tile([128, 512], F32, tag="h1", bufs=2,
                                  name="h1")
                    nc.scalar.activation(h1[:].bitcast(F32R), pre[:],
                                         ACTF.Relu, bias=b2a[:])
                    h2 = pse.tile([128, 512], F32, tag="h2", name="h2_ps")
                    nc.tensor.matmul(h2[:], _r(w2b[:]), _r(h1[:]))
                    nc.vector.tensor_reduce(
                        out=x2T[:, nsl].bitcast(F32R),
                        in_=h2[:].rearrange("c (n k) -> c n k", k=K),
                        axis=AX.X, op=OP.max)
                nc.vector.tensor_scalar(
                    x2T[:, tsl].bitcast(F32R), x2T[:, tsl], b2b[:], None,
                    op0=OP.add)

            prev = None
            for g in range(4):
                xg = l2_heads(g)
                if prev is not None:
                    pg, pxg = prev
                    for q in range(4):
                        l2_tail(4 * pg + q, q, pxg)
                prev = (g, xg)
            pg, pxg = prev
            for q in range(4):
                l2_tail(4 * pg + q, q, pxg)

            if dbg_stop == 2:
                nc.sync.dma_start(out_d.ap().rearrange("n c -> c n"),
                                  x2T[0:2, :])
                return

            # ---------------- layer 3 ----------------
            x2hi_f = scr.tile([64, N], F32, tag="xsq", bufs=1, name="x2hi_f")
            nc.sync.dma_start(x2hi_f[:], x2T[64:128, :])
            pk3 = x2pk[0:64, :].bitcast(BF16).rearrange("c (n t) -> c n t",
                                                        t=2)
            nc.scalar.activation(pk3[:, :, 0:1], x2T[0:64, :].unsqueeze(-1),
                                 ACTF.Copy)
            nc.scalar.activation(pk3[:, :, 1:2], x2hi_f[:].unsqueeze(-1),
                                 ACTF.Copy)
            nc.sync.dma_start(x2pk[64:128, :], x2pk[0:64, :])

            twox3, negsq3 = layer_prep("l3", x2T, 128, False)

            def l3_heads(g):
                iu_cat = scr.tile([128, 128], U16, tag="iu_cat",
                                  name="iu_cat")
                for h in range(2):
                    t = 2 * g + h

                    def sink(v_u32, h=h):
                        iu32 = extract_iu32(v_u32)
                        nc.vector.tensor_copy(
                            iu_cat[:, 64 * h:64 * h + 64]
                            .rearrange("p (r k) -> p r k", r=4),
                            iu32[:].unsqueeze(1).to_broadcast([128, 4, 16]))

                    tile_head(t, x2T, 128, twox3, negsq3, False, sink)
                idx16 = scr.tile([128, 128], U16, tag="idx16", name="idx16")
                nc.sync.dma_start_transpose(idx16[:], iu_cat[:])
                xg = scr.tile([128, N], F32, tag="xg0", name="xg0")
                nc.gpsimd.ap_gather(
                    out_ap=xg[:], in_ap=x2pk[:],
                    idxs_ap=idx16[:].bitcast(I16),
                    channels=128, num_elems=N, d=1, num_idxs=N)
                return xg

            def l3_tail(t, h, xg):
                hsl = slice(64 * h, 64 * h + 64)
                ts0 = t * 128
                tsl = slice(ts0, ts0 + 128)
                mo = _chunks(256)
                xgv = xg[hsl, :].bitcast(BF16).rearrange("c (n t) -> c n t",
                                                         t=2)
                xglo = scr.tile([128, N], BF16, tag="xglo", bufs=1,
                                name="xglo")
                xghi = scr.tile([128, N], BF16, tag="xghi", bufs=1,
                                name="xghi")
                nc.scalar.activation(xglo[hsl, :].unsqueeze(-1),
                                     xgv[:, :, 0:1], ACTF.Copy)
                nc.scalar.activation(xghi[hsl, :].unsqueeze(-1),
                                     xgv[:, :, 1:2], ACTF.Copy)
                for j in range(4):
                    fsl = slice(j * 512, (j + 1) * 512)
                    nsl = slice(ts0 + j * 32, ts0 + (j + 1) * 32)
                    h1cs = []
                    for mi, (ms, mp) in enumerate(mo):
                        pre = pse.tile([128, 512], F32, tag="pre",
                                       name="pre_ps")
                        nc.tensor.matmul(pre[:mp, :],
                                         wd3lo_b[hsl, ms:ms + mp],
                                         xglo[hsl, fsl],
                                         start=True, stop=False)
                        nc.tensor.matmul(pre[:mp, :],
                                         wd3hi_b[hsl, ms:ms + mp],
                                         xghi[hsl, fsl],
                                         start=False, stop=False)
                        nc.tensor.matmul(
                            pre[:mp, :], _r(wdiff3[:, ms:ms + mp]),
                            _r(x2T[:, nsl].unsqueeze(-1)
                               .to_broadcast([128, 32, K])),
                            start=False, stop=True)
                        h1c = scr.tile([128, 512], F32, tag=f"h1_{mi}",
                                       bufs=1, name="h1c")
                        nc.scalar.activation(h1c[:mp, :].bitcast(F32R),
                                             pre[:mp, :], ACTF.Relu,
                                             bias=b3a[mi][:])
                        h1cs.append(h1c)
                    for gi, (gs, gp2) in enumerate(mo):
                        h2 = pse.tile([128, 512], F32, tag="h2", name="h2_ps")
                        for mi, (ms, mp) in enumerate(mo):
                            nc.tensor.matmul(
                                h2[:gp2, :], _r(w3b[mi][:, gs:gs + gp2]),
                                _r(h1cs[mi][:mp, :]),
                                start=(mi == 0), stop=(mi == len(mo) - 1))
                        fo = x3Ta if gi == 0 else x3Tb
                        nc.vector.tensor_reduce(
                            out=fo[:, nsl].bitcast(F32R),
                            in_=h2[:gp2, :].rearrange("c (n k) -> c n k",
                                                      k=K),
                            axis=AX.X, op=OP.max)
                for gi in range(2):
                    fo = x3Ta if gi == 0 else x3Tb
                    nc.vector.tensor_scalar(
                        fo[:, tsl].bitcast(F32R), fo[:, tsl], b3b[gi][:],
                        None, op0=OP.add)

            prev = None
            for g in range(8):
                xg = l3_heads(g)
                if prev is not None:
                    pg, pxg = prev
                    for h in range(2):
                        l3_tail(2 * pg + h, h, pxg)
                prev = (g, xg)
            pg, pxg = prev
            for h in range(2):
                l3_tail(2 * pg + h, h, pxg)

            if dbg_stop == 3:
                nc.sync.dma_start(out_d.ap().rearrange("n c -> c n"),
                                  x3Ta[0:2, :])
                return

            # ---------------- final MLP ----------------
            featc = [x1T[0:64, :], x2T, x3Ta, x3Tb]

            for j in range(4):
                fsl = slice(j * 512, (j + 1) * 512)
                h1c = [scr.tile([128, 512], F32, tag=f"mh1_{m}", bufs=1,
                                name=f"mh1_{m}") for m in range(4)]
                for m in range(4):
                    ps = pse.tile([128, 512], F32, tag="pre")
                    for ci, wc in enumerate(m1):
                        nc.tensor.matmul(
                            ps[:], _r(wc[:, m * 128:(m + 1) * 128]),
                            _r(featc[ci][:, fsl]),
                            start=(ci == 0), stop=(ci == 3))
                    nc.scalar.activation(h1c[m][:].bitcast(F32R), ps[:],
                                         ACTF.Relu, bias=bm1[m][:])
                h2c = [scr.tile([128, 512], F32, tag=f"mh2_{m}", bufs=1,
                                name=f"mh2_{m}") for m in range(4)]
                for m in range(4):
                    ps = pse.tile([128, 512], F32, tag="pre")
                    for ci in range(4):
                        nc.tensor.matmul(
                            ps[:], _r(m2[ci][:, m * 128:(m + 1) * 128]),
                            _r(h1c[ci][:]),
                            start=(ci == 0), stop=(ci == 3))
                    nc.scalar.activation(h2c[m][:].bitcast(F32R), ps[:],
                                         ACTF.Relu, bias=bm2[m][:])
                h3c = [scr.tile([128, 512], F32, tag=f"mh3_{m}", bufs=1,
                                name=f"mh3_{m}") for m in range(2)]
                for m in range(2):
                    ps = pse.tile([128, 512], F32, tag="pre")
                    for ci in range(4):
                        nc.tensor.matmul(
                            ps[:], _r(m3[ci][:, m * 128:(m + 1) * 128]),
                            _r(h2c[ci][:]),
                            start=(ci == 0), stop=(ci == 3))
                    nc.scalar.activation(h3c[m][:], ps[:], ACTF.Relu,
                                         bias=bm3[m][:])

                for st in range(4):
                    t0 = j * 512 + st * 128
                    tsl = slice(t0, t0 + 128)
                    lsl = slice(st * 128, (st + 1) * 128)
                    o_ps = psi.tile([128, 2], F32, tag="tmp")
                    nc.tensor.matmul(o_ps[:], h3c[0][:, lsl], m4[0][:],
                                     start=True, stop=False)
                    nc.tensor.matmul(o_ps[:], h3c[1][:, lsl], m4[1][:],
                                     start=False, stop=False)
                    nc.tensor.matmul(o_ps[:], ones1[:, tsl], bm4[:],
                                     start=False, stop=True)
                    mx = scr.tile([128, 1], F32, tag="mx")
                    nc.vector.tensor_reduce(out=mx[:], in_=o_ps[:], axis=AX.X,
                                            op=OP.max)
                    hm = scr.tile([128, 2], F32, tag="hm")
                    nc.vector.tensor_scalar(hm[:], o_ps[:], mx[:], None,
                                            op0=OP.subtract)
                    ex = scr.tile([128, 2], F32, tag="ex")
                    ssum = scr.tile([128, 1], F32, tag="ssum")
                    nc.scalar.activation(ex[:], hm[:], ACTF.Exp,
                                         accum_out=ssum[:])
                    lns = scr.tile([128, 1], F32, tag="lns")
                    nc.scalar.activation(lns[:], ssum[:], ACTF.Ln)
                    res = scr.tile([128, 2], F32, tag="res")
                    nc.vector.tensor_scalar(res[:], hm[:], lns[:], None,
                                            op0=OP.subtract)
                    nc.sync.dma_start(out_d.ap()[tsl, :], res[:])


_NC_CACHE = {}


def _get_nc(repeat_loop=False):
    key = repeat_loop
    if key not in _NC_CACHE:
        _NC_CACHE[key] = build_nc(repeat_loop)
    return _NC_CACHE[key]


def kernel(**inputs):
    nc = _get_nc()
    in_maps = []
    for g in range(B):
        m = {"x": np.ascontiguousarray(np.asarray(inputs["x"][g], np.float32))}
        for w in WEIGHT_NAMES:
            m[w] = np.ascontiguousarray(np.asarray(inputs[w], np.float32))
        in_maps.append(m)
    res = bass_utils.run_bass_kernel_spmd(nc, in_maps, core_ids=list(range(B)))
    return np.stack([res.results[g]["out"] for g in range(B)], axis=0)



# revision 31
# speedup vs baseline: 4.3505x; 4.3505x over previous
"""DGCNN (3x DynamicEdgeConv + MLP head) Trainium2 Bass kernel.

Data-parallel over the batch axis: 8 graphs -> 8 NeuronCores, weights
replicated. Per graph (N=2048 nodes, K=16 neighbors):
  - kNN scores s'[i,j] = 2<x_i,x_j> - |x_j|^2 via exact fp32 PE matmuls
    (ones/negsq row folded into the contraction for L1/L2)
  - top-16 via index-packed scores: the low 11 mantissa bits of each
    score are replaced by the column index (one DVE pass), then
    max8 / match_replace / max8 yield values whose low bits ARE the
    neighbor indices (no max_index passes, no duplicate hazards)
  - neighbor gather with gpsimd ap_gather; each 16-partition Q7 core
    processes its own tile's index list, so one gather instruction
    serves 8 tiles (L1: scalar features replicated on all partitions),
    4 tiles (L2: bf16-pair packed 32-row features, 4 replicas) or
    2 tiles (L3: bf16-pair packed 64-row features, 2 replicas)
  - edge MLP first linear on PE from gathered x_j (bf16) plus broadcast
    x_i term (fp32r), relu on ACT, second linear fp32r, max-pool over k
    via DVE segmented reduce; groups software-pipelined so PE/DVE work
    overlaps the Pool-engine gathers
  - final MLP in [C, N] layout fp32r, classifier + log_softmax
"""

import numpy as np

import concourse.bacc as bacc
import concourse.mybir as mybir
from concourse import tile
from concourse import bass_utils

F32 = mybir.dt.float32
F32R = mybir.dt.float32r
BF16 = mybir.dt.bfloat16
U16 = mybir.dt.uint16
U32 = mybir.dt.uint32
I16 = mybir.dt.int16
AX = mybir.AxisListType
OP = mybir.AluOpType
ACTF = mybir.ActivationFunctionType


def _r(ap):
    """fp32 -> fp32r view: 4x PE throughput for matmuls >=256 cols wide."""
    return ap.bitcast(F32R)


B, N, K = 8, 2048, 16
NT = N // 128
V_GPSIMD_ELEM = True     # twox/xsq/bias adds on Pool (else DVE)
NEG = -3.0e38
MASK_HI = 0xFFFFF800
MASK_LO = 0x000007FF

WEIGHT_NAMES = [
    "w1a_w", "w1a_b", "w1b_w", "w1b_b",
    "w2a_w", "w2a_b", "w2b_w", "w2b_b",
    "w3a_w", "w3a_b", "w3b_w", "w3b_b",
    "m1_w", "m1_b", "m2_w", "m2_b", "m3_w", "m3_b", "m4_w", "m4_b",
]
WEIGHT_SHAPES = {
    "w1a_w": (2, 64), "w1a_b": (64,), "w1b_w": (64, 64), "w1b_b": (64,),
    "w2a_w": (128, 128), "w2a_b": (128,), "w2b_w": (128, 128), "w2b_b": (128,),
    "w3a_w": (256, 256), "w3a_b": (256,), "w3b_w": (256, 256), "w3b_b": (256,),
    "m1_w": (448, 512), "m1_b": (512,), "m2_w": (512, 512), "m2_b": (512,),
    "m3_w": (512, 256), "m3_b": (256,), "m4_w": (256, 2), "m4_b": (2,),
}


def _chunks(n, c=128):
    return [(s, min(c, n - s)) for s in range(0, n, c)]


def build_nc(repeat_loop=False, dbg_stop=None):
    nc = bacc.Bacc("TRN2", target_bir_lowering=False, debug=False)

    x_d = nc.dram_tensor("x", [N, 1], F32, kind="ExternalInput")
    w_d = {
        n: nc.dram_tensor(n, list(WEIGHT_SHAPES[n]), F32, kind="ExternalInput")
        for n in WEIGHT_NAMES
    }
    steps_d = None
    if repeat_loop:
        steps_d = nc.dram_tensor("steps", [1, 1], mybir.dt.uint32,
                                 kind="ExternalInput")
    out_d = nc.dram_tensor("out", [N, 2], F32, kind="ExternalOutput")

    with tile.TileContext(nc) as tc:
        _emit(nc, tc, x_d, w_d, out_d, steps_d, dbg_stop)
    nc.compile()
    return nc


def _emit(nc, tc, x_d, w_d, out_d, steps_d, dbg_stop=None):
    from contextlib import ExitStack

    ctx = ExitStack()
    with ctx:
        const = ctx.enter_context(tc.tile_pool(name="const", bufs=1))
        wpool = ctx.enter_context(tc.tile_pool(name="wpool", bufs=1))
        feats = ctx.enter_context(tc.tile_pool(name="feats", bufs=1))

        # ---- constants ----
        ones1 = const.tile([1, N], F32)
        nc.vector.memset(ones1[:], 1.0)
        onesc = const.tile([128, 1], F32)
        nc.vector.memset(onesc[:], 1.0)
        iota_u = const.tile([128, N], U32)
        nc.gpsimd.iota(iota_u[:], [[1, N]], channel_multiplier=0)
        mhi = const.tile([128, 1], U32)
        nc.gpsimd.memset(mhi[:], MASK_HI)
        mlo = const.tile([128, 1], U32)
        nc.gpsimd.memset(mlo[:], MASK_LO)

        # ---- load weights ----
        def load2d(name, row_splits=None, cast_r=False):
            rows, cols = WEIGHT_SHAPES[name]
            ts = []
            splits = row_splits or _chunks(rows)
            for i, (s, p) in enumerate(splits):
                t = wpool.tile([p, cols], F32, tag=f"w_{name}_{i}",
                               name=f"w_{name}_{i}")
                if cast_r:
                    nc.gpsimd.dma_start(t[:].bitcast(F32R),
                                        w_d[name].ap()[s:s + p, :])
                else:
                    nc.sync.dma_start(t[:], w_d[name].ap()[s:s + p, :])
                ts.append(t)
            return ts

        def load_bias_col(name):
            (n_,) = WEIGHT_SHAPES[name]
            ts = []
            for i, (s, p) in enumerate(_chunks(n_)):
                t = wpool.tile([p, 1], F32, tag=f"b_{name}_{i}",
                               name=f"b_{name}_{i}")
                nc.sync.dma_start(t[:], w_d[name].ap()[s:s + p].unsqueeze(-1))
                ts.append(t)
            return ts

        w1a = load2d("w1a_w")[0]                 # [2, 64]
        w1b = load2d("w1b_w", cast_r=True)[0]    # [64, 64]
        w2a = load2d("w2a_w")[0]                 # [128, 128]
        w2b = load2d("w2b_w", cast_r=True)[0]    # [128, 128]
        w3a = load2d("w3a_w")                    # 2 x [128, 256]
        w3b = load2d("w3b_w", cast_r=True)       # 2 x [128, 256]
        m1 = load2d("m1_w", row_splits=[(0, 64), (64, 128), (192, 128),
                                        (320, 128)], cast_r=True)
        m2 = load2d("m2_w", cast_r=True)
        m3 = load2d("m3_w", cast_r=True)
        m4 = load2d("m4_w")
        b1a = load_bias_col("w1a_b")[0]
        b1b = load_bias_col("w1b_b")[0]
        b2a = load_bias_col("w2a_b")[0]
        b2b = load_bias_col("w2b_b")[0]
        b3a = load_bias_col("w3a_b")
        b3b = load_bias_col("w3b_b")
        bm1 = load_bias_col("m1_b")
        bm2 = load_bias_col("m2_b")
        bm3 = load_bias_col("m3_b")
        bm4 = wpool.tile([1, 2], F32)
        nc.sync.dma_start(bm4[:], w_d["m4_b"].ap().unsqueeze(0))

        # wdiff = Wx - Wd (first C_in rows minus last C_in rows of w*a)
        wd1 = wpool.tile([1, 64], F32)
        nc.sync.dma_start(wd1[:], w_d["w1a_w"].ap()[1:2, :])
        wdiff1 = wpool.tile([1, 64], F32)
        nc.vector.tensor_tensor(wdiff1[:], w1a[0:1, :], wd1[:],
                                op=OP.subtract)
        # E_all[:, 64*tp : 64*(tp+1)] is wd1 placed on partition row 16*tp,
        # zero elsewhere: matmul(E_all-slice, xg) extracts tile tp's gathered
        # scalars (parked on core tp's partitions) as a rank-1 xj term.
        E_all = wpool.tile([128, 512], F32)
        nc.vector.memset(E_all[:], 0.0)
        for tp in range(8):
            nc.sync.dma_start(E_all[16 * tp:16 * tp + 1,
                                    64 * tp:64 * tp + 64], wd1[:])
        wd2 = wpool.tile([64, 128], F32)
        nc.sync.dma_start(wd2[:], w_d["w2a_w"].ap()[64:128, :])
        wdiff2 = wpool.tile([64, 128], F32)
        nc.vector.tensor_tensor(wdiff2[:].bitcast(F32R), w2a[0:64, :], wd2[:],
                                op=OP.subtract)
        wdiff3 = wpool.tile([128, 256], F32)
        nc.vector.tensor_tensor(wdiff3[:].bitcast(F32R), w3a[0][:], w3a[1][:],
                                op=OP.subtract)

        # L2 gather weights: wd2 bf16 halves (x1 packed pairs (c, c+32)),
        # replicated at partitions 0/32/64/96
        wd2lo_b = wpool.tile([128, 128], BF16)
        wd2hi_b = wpool.tile([128, 128], BF16)
        nc.scalar.activation(wd2lo_b[0:32, :], wd2[0:32, :], ACTF.Copy)
        nc.scalar.activation(wd2hi_b[0:32, :], wd2[32:64, :], ACTF.Copy)
        for q in range(1, 4):
            nc.sync.dma_start(wd2lo_b[32 * q:32 * q + 32, :], wd2lo_b[0:32, :])
            nc.sync.dma_start(wd2hi_b[32 * q:32 * q + 32, :], wd2hi_b[0:32, :])

        # L3 gather weights: wd3 bf16 halves replicated at partitions 0/64
        wd3hi_f = wpool.tile([64, 256], F32)
        nc.sync.dma_start(wd3hi_f[:], w_d["w3a_w"].ap()[192:256, :])
        wd3lo_b = wpool.tile([128, 256], BF16)
        wd3hi_b = wpool.tile([128, 256], BF16)
        nc.scalar.activation(wd3lo_b[0:64, :], w3a[1][0:64, :], ACTF.Copy)
        nc.scalar.activation(wd3hi_b[0:64, :], wd3hi_f[:], ACTF.Copy)
        nc.sync.dma_start(wd3lo_b[64:128, :], wd3lo_b[0:64, :])
        nc.sync.dma_start(wd3hi_b[64:128, :], wd3hi_b[0:64, :])

        # ---- feature tensors ----
        x0T = feats.tile([2, N], F32)      # row 0 = x, row 1 = ones (merged)
        nc.sync.dma_start(x0T[0:1, :], x_d.ap().rearrange("n 1 -> 1 n"))
        nc.sync.dma_start(x0T[1:2, :].bitcast(F32R),
                  ones1[0:1, :].bitcast(F32R))
        x0rep = feats.tile([128, N], F32)  # x scalars on every partition
        nc.sync.dma_start(x0rep[0:1, :].bitcast(F32R),
                  x0T[0:1, :].bitcast(F32R))
        _p = 1
        while _p < 128:
            nc.sync.dma_start(x0rep[_p:2 * _p, :], x0rep[0:_p, :])
            _p *= 2

        x1T = feats.tile([65, N], F32)     # rows 0:64 feats, row 64 = ones
        nc.vector.memset(x1T[64:65, :], 1.0)
        x1pk = feats.tile([128, N], F32)
        x2T = feats.tile([128, N], F32)
        x2pk = feats.tile([128, N], F32)
        x3Ta = feats.tile([128, N], F32)
        x3Tb = feats.tile([128, N], F32)

        body_ctx = ExitStack()
        with body_ctx:
            scr = body_ctx.enter_context(tc.tile_pool(name="scr", bufs=2))
            pss = body_ctx.enter_context(
                tc.tile_pool(name="pss", bufs=2, space="PSUM"))
            psi = body_ctx.enter_context(
                tc.tile_pool(name="psi", bufs=2, space="PSUM"))
            pse = body_ctx.enter_context(
                tc.tile_pool(name="pse", bufs=2, space="PSUM"))

            if steps_d is not None:
                steps_sb = const.tile([1, 1], mybir.dt.uint32)
                nc.sync.dma_start(steps_sb[:], steps_d.ap())
                (_, (steps_val,)) = nc.values_load_multi_w_load_instructions(
                    steps_sb[:], min_val=1, max_val=1000000,
                    skip_runtime_bounds_check=True)
                body_ctx.enter_context(tc.For_i(0, steps_val, 1))

            def layer_prep(lname, feat_lhs, c_in, merged, use_r=True):
                """2x rows + xsq + negsq. merged: negsq goes into row c_in
                of the twox tile so the head needs one matmul. use_r=False
                keeps the full-fp32 score path (L1: 1-D kNN gaps are below
                fp32r's mantissa)."""
                rw = _r if use_r else (lambda ap: ap)
                twox = scr.tile([128, N], F32, tag="twox", bufs=1,
                                name=f"twox_{lname}")
                eng = nc.gpsimd if (c_in >= 32 and V_GPSIMD_ELEM) else nc.vector
                eng.tensor_scalar_mul(rw(twox[:c_in, :]),
                                      feat_lhs[:c_in, :], 2.0)
                xsq = scr.tile([128, N], F32, tag="xsq", bufs=1,
                               name=f"xsq_{lname}")
                # engines can only write partition starts 0/32/64/96: a
                # merged negsq row at an unaligned partition (L1: row 1)
                # is staged in xsq row 0 and DMA'd into place.
                aligned = merged and c_in % 32 == 0
                if aligned:
                    negsq = twox[c_in:c_in + 1, :]
                else:
                    # row 0 of xsq doubles as negsq home: each chunk's
                    # negsq write lands after the PE consumed that chunk
                    negsq = xsq[0:1, :]
                for j in range(4):
                    fsl = slice(j * 512, (j + 1) * 512)
                    sq_ps = psi.tile([1, 512], F32, tag="tmp")
                    eng.tensor_tensor(
                        rw(xsq[:c_in, fsl]), feat_lhs[:c_in, fsl],
                        feat_lhs[:c_in, fsl], op=OP.mult)
                    nc.tensor.matmul(
                        sq_ps[:], rw(onesc[:c_in, :]), rw(xsq[:c_in, fsl]),
                        start=True, stop=True)
                    nc.scalar.activation(rw(negsq[:, fsl]),
                                         sq_ps[:], ACTF.Copy, scale=-1.0)
                    if merged and not aligned:
                        nc.sync.dma_start(
                            rw(twox[c_in:c_in + 1, fsl]),
                            rw(xsq[0:1, fsl]))
                return twox, negsq

            def tile_head(t, feat_lhs, c_in, twox, negsq, merged, iu_sink,
                          use_r=True):
                """s'-matmul + packed top-16 for node tile t."""
                ts0 = t * 128
                tsl = slice(ts0, ts0 + 128)
                rows = c_in + 1 if merged else c_in
                pk = scr.tile([128, N], F32, tag="pk", bufs=1, name="pk")
                for j in range(4):
                    fsl = slice(j * 512, (j + 1) * 512)
                    s_ps = pss.tile([128, 512], F32, tag="s_ps", name="s_ps")
                    rr = _r if use_r else (lambda ap: ap)
                    if merged:
                        nc.tensor.matmul(
                            s_ps[:], rr(feat_lhs[:rows, tsl]),
                            rr(twox[:rows, fsl]), start=True, stop=True)
                    else:
                        nc.tensor.matmul(
                            s_ps[:], rr(feat_lhs[:c_in, tsl]),
                            rr(twox[:c_in, fsl]), start=True, stop=False)
                        nc.tensor.matmul(
                            s_ps[:], rr(ones1[:, tsl]), rr(negsq[:, fsl]),
                            start=False, stop=True)
                    nc.vector.scalar_tensor_tensor(
                        pk[:, fsl].bitcast(U32), s_ps[:].bitcast(U32), mhi[:],
                        iota_u[:, fsl], op0=OP.bitwise_and, op1=OP.bitwise_or)
                v = scr.tile([128, 16], F32, tag="v16", name="v16")
                nc.vector.max(out=v[:, 0:8], in_=pk[:])
                nc.vector.match_replace(out=pk[:], in_to_replace=v[:, 0:8],
                                        in_values=pk[:], imm_value=NEG)
                nc.vector.max(out=v[:, 8:16], in_=pk[:])
                iu_sink(v[:].bitcast(U32))

            def extract_iu32(v_u32):
                iu32 = scr.tile([128, 16], U32, tag="iu32", name="iu32")
                nc.vector.tensor_scalar(iu32[:], v_u32, mlo[:], None,
                                        op0=OP.bitwise_and)
                return iu32

            # ---------------- layer 1 ----------------
            twox1, _ = layer_prep("l1", x0T, 1, True, use_r=False)

            def l1_heads(g):
                iu_cat = scr.tile([128, 128], U16, tag="iu_cat",
                                  name="iu_cat")
                for tp in range(8):
                    t = 8 * g + tp

                    def sink(v_u32, tp=tp):
                        iu32 = extract_iu32(v_u32)
                        nc.vector.tensor_copy(
                            iu_cat[:, 16 * tp:16 * tp + 16], iu32[:])

                    tile_head(t, x0T, 1, twox1, None, True, sink,
                              use_r=False)
                idx16 = scr.tile([128, 128], U16, tag="idx16", name="idx16")
                nc.sync.dma_start_transpose(idx16[:], iu_cat[:])
                xg = scr.tile([128, N], F32, tag="xg0", name="xg0")
                nc.gpsimd.ap_gather(
                    out_ap=xg[:], in_ap=x0rep[:],
                    idxs_ap=idx16[:].bitcast(I16),
                    channels=128, num_elems=N, d=1, num_idxs=N)
                return xg

            def l1_tail(t, xg):
                tp = t % 8
                ts0 = t * 128
                tsl = slice(ts0, ts0 + 128)
                esl = slice(64 * tp, 64 * tp + 64)
                for j in range(4):
                    fsl = slice(j * 512, (j + 1) * 512)
                    nsl = slice(ts0 + j * 32, ts0 + (j + 1) * 32)
                    pre = pse.tile([128, 512], F32, tag="pre", name="pre_ps")
                    nc.tensor.matmul(pre[0:64, :], E_all[:, esl],
                                     xg[:, fsl], start=True, stop=False)
                    nc.tensor.matmul(
                        pre[0:64, :], wdiff1[:],
                        x0T[0:1, nsl].unsqueeze(-1).to_broadcast([1, 32, K]),
                        start=False, stop=True)
                    h1 = scr.tile([128, 512], F32, tag="h1", bufs=2,
                                  name="h1")
                    nc.scalar.activation(h1[0:64, :].bitcast(F32R),
                                         pre[0:64, :], ACTF.Relu,
                                         bias=b1a[:])
                    h2 = pse.tile([128, 512], F32, tag="h2", name="h2_ps")
                    nc.tensor.matmul(h2[0:64, :], _r(w1b[:]), _r(h1[0:64, :]))
                    nc.vector.tensor_reduce(
                        out=x1T[0:64, nsl].bitcast(F32R),
                        in_=h2[0:64, :].rearrange("c (n k) -> c n k", k=K),
                        axis=AX.X, op=OP.max)
                ev = nc.gpsimd if V_GPSIMD_ELEM else nc.vector
                ev.tensor_scalar(
                    x1T[0:64, tsl].bitcast(F32R), x1T[0:64, tsl], b1b[:],
                    None, op0=OP.add)

            prev = None
            for g in range(2):
                xg = l1_heads(g)
                if prev is not None:
                    pg, pxg = prev
                    for tp in range(8):
                        l1_tail(8 * pg + tp, pxg)
                prev = (g, xg)
            pg, pxg = prev
            for tp in range(8):
                l1_tail(8 * pg + tp, pxg)

            if dbg_stop == 1:
                nc.sync.dma_start(out_d.ap().rearrange("n c -> c n"),
                                  x1T[0:2, :])
                return

            # ---------------- layer 2 ----------------
            pk2 = x1pk[0:32, :].bitcast(BF16).rearrange("c (n t) -> c n t",
                                                        t=2)
            nc.scalar.activation(pk2[:, :, 0:1], x1T[0:32, :].unsqueeze(-1),
                                 ACTF.Copy)
            nc.scalar.activation(pk2[:, :, 1:2], x1T[32:64, :].unsqueeze(-1),
                                 ACTF.Copy)
            for q in range(1, 4):
                nc.sync.dma_start(x1pk[32 * q:32 * q + 32, :], x1pk[0:32, :])

            twox2, _ = layer_prep("l2", x1T, 64, True)

            def l2_heads(g):
                iu_cat = scr.tile([128, 128], U16, tag="iu_cat",
                                  name="iu_cat")
                for q in range(4):
                    t = 4 * g + q

                    def sink(v_u32, q=q):
                        iu32 = extract_iu32(v_u32)
                        nc.vector.tensor_copy(
                            iu_cat[:, 32 * q:32 * q + 32]
                            .rearrange("p (r k) -> p r k", r=2),
                            iu32[:].unsqueeze(1).to_broadcast([128, 2, 16]))

                    tile_head(t, x1T, 64, twox2, None, True, sink)
                idx16 = scr.tile([128, 128], U16, tag="idx16", name="idx16")
                nc.sync.dma_start_transpose(idx16[:], iu_cat[:])
                xg = scr.tile([128, N], F32, tag="xg0", name="xg0")
                nc.gpsimd.ap_gather(
                    out_ap=xg[:], in_ap=x1pk[:],
                    idxs_ap=idx16[:].bitcast(I16),
                    channels=128, num_elems=N, d=1, num_idxs=N)
                return xg

            def l2_tail(t, q, xg):
                if q == 3:
                    # PE matmul base partitions must be 0/32/64: quarter 3
                    # (base 96) is copied down and processed at base 0.
                    xg3 = scr.tile([32, N], F32, tag="xg3b", bufs=1,
                                   name="xg3b")
                    nc.sync.dma_start(xg3[:], xg[96:128, :])
                    xg, q = xg3, 0
                    qsl = slice(0, 32)
                else:
                    qsl = slice(32 * q, 32 * q + 32)
                ts0 = t * 128
                tsl = slice(ts0, ts0 + 128)
                xgv = xg[qsl, :].bitcast(BF16).rearrange("c (n t) -> c n t",
                                                         t=2)
                for j in range(4):
                    fsl = slice(j * 512, (j + 1) * 512)
                    nsl = slice(ts0 + j * 32, ts0 + (j + 1) * 32)
                    pre = pse.tile([128, 512], F32, tag="pre", name="pre_ps")
                    nc.tensor.matmul(pre[:], wd2lo_b[qsl, :],
                                     xgv[:, fsl, 0:1],
                                     start=True, stop=False)
                    nc.tensor.matmul(pre[:], wd2hi_b[qsl, :],
                                     xgv[:, fsl, 1:2],
                                     start=False, stop=False)
                    nc.tensor.matmul(
                        pre[:], _r(wdiff2[:]),
                        _r(x1T[0:64, nsl].unsqueeze(-1)
                           .to_broadcast([64, 32, K])),
                        start=False, stop=True)
                    h1 = scr.tile([128, 512], F32, tag="h1", bufs=2,
                                  name="h1")
                    nc.scalar.activation(h1[:].bitcast(F32R), pre[:],
                                         ACTF.Relu, bias=b2a[:])
                    h2 = pse.tile([128, 512], F32, tag="h2", name="h2_ps")
                    nc.tensor.matmul(h2[:], _r(w2b[:]), _r(h1[:]))
                    nc.vector.tensor_reduce(
                        out=x2T[:, nsl].bitcast(F32R),
                        in_=h2[:].rearrange("c (n k) -> c n k", k=K),
                        axis=AX.X, op=OP.max)
                ev = nc.gpsimd if V_GPSIMD_ELEM else nc.vector
                ev.tensor_scalar(
                    x2T[:, tsl].bitcast(F32R), x2T[:, tsl], b2b[:], None,
                    op0=OP.add)

            prev = None
            for g in range(4):
                xg = l2_heads(g)
                if prev is not None:
                    pg, pxg = prev
                    for q in range(4):
                        l2_tail(4 * pg + q, q, pxg)
                prev = (g, xg)
            pg, pxg = prev
            for q in range(4):
                l2_tail(4 * pg + q, q, pxg)

            if dbg_stop == 2:
                nc.sync.dma_start(out_d.ap().rearrange("n c -> c n"),
                                  x2T[0:2, :])
                return

            # ---------------- layer 3 ----------------
            x2hi_f = scr.tile([64, N], F32, tag="xsq", bufs=1, name="x2hi_f")
            nc.sync.dma_start(x2hi_f[:], x2T[64:128, :])
            pk3 = x2pk[0:64, :].bitcast(BF16).rearrange("c (n t) -> c n t",
                                                        t=2)
            nc.scalar.activation(pk3[:, :, 0:1], x2T[0:64, :].unsqueeze(-1),
                                 ACTF.Copy)
            nc.scalar.activation(pk3[:, :, 1:2], x2hi_f[:].unsqueeze(-1),
                                 ACTF.Copy)
            nc.sync.dma_start(x2pk[64:128, :], x2pk[0:64, :])

            twox3, negsq3 = layer_prep("l3", x2T, 128, False)

            def l3_heads(g):
                iu_cat = scr.tile([128, 128], U16, tag="iu_cat",
                                  name="iu_cat")
                for h in range(2):
                    t = 2 * g + h

                    def sink(v_u32, h=h):
                        iu32 = extract_iu32(v_u32)
                        nc.vector.tensor_copy(
                            iu_cat[:, 64 * h:64 * h + 64]
                            .rearrange("p (r k) -> p r k", r=4),
                            iu32[:].unsqueeze(1).to_broadcast([128, 4, 16]))

                    tile_head(t, x2T, 128, twox3, negsq3, False, sink)
                idx16 = scr.tile([128, 128], U16, tag="idx16", name="idx16")
                nc.sync.dma_start_transpose(idx16[:], iu_cat[:])
                xg = scr.tile([128, N], F32, tag="xg0", name="xg0")
                nc.gpsimd.ap_gather(
                    out_ap=xg[:], in_ap=x2pk[:],
                    idxs_ap=idx16[:].bitcast(I16),
                    channels=128, num_elems=N, d=1, num_idxs=N)
                return xg

            def l3_tail(t, h, xg):
                hsl = slice(64 * h, 64 * h + 64)
                ts0 = t * 128
                tsl = slice(ts0, ts0 + 128)
                mo = _chunks(256)
                xgv = xg[hsl, :].bitcast(BF16).rearrange("c (n t) -> c n t",
                                                         t=2)
                for j in range(4):
                    fsl = slice(j * 512, (j + 1) * 512)
                    nsl = slice(ts0 + j * 32, ts0 + (j + 1) * 32)
                    h1cs = []
                    for mi, (ms, mp) in enumerate(mo):
                        pre = pse.tile([128, 512], F32, tag="pre",
                                       name="pre_ps")
                        nc.tensor.matmul(pre[:mp, :],
                                         wd3lo_b[hsl, ms:ms + mp],
                                         xgv[:, fsl, 0:1],
                                         start=True, stop=False)
                        nc.tensor.matmul(pre[:mp, :],
                                         wd3hi_b[hsl, ms:ms + mp],
                                         xgv[:, fsl, 1:2],
                                         start=False, stop=False)
                        nc.tensor.matmul(
                            pre[:mp, :], _r(wdiff3[:, ms:ms + mp]),
                            _r(x2T[:, nsl].unsqueeze(-1)
                               .to_broadcast([128, 32, K])),
                            start=False, stop=True)
                        h1c = scr.tile([128, 512], F32, tag=f"h1_{mi}",
                                       bufs=1, name="h1c")
                        nc.scalar.activation(h1c[:mp, :].bitcast(F32R),
                                             pre[:mp, :], ACTF.Relu,
                                             bias=b3a[mi][:])
                        h1cs.append(h1c)
                    for gi, (gs, gp2) in enumerate(mo):
                        h2 = pse.tile([128, 512], F32, tag="h2", name="h2_ps")
                        for mi, (ms, mp) in enumerate(mo):
                            nc.tensor.matmul(
                                h2[:gp2, :], _r(w3b[mi][:, gs:gs + gp2]),
                                _r(h1cs[mi][:mp, :]),
                                start=(mi == 0), stop=(mi == len(mo) - 1))
                        fo = x3Ta if gi == 0 else x3Tb
                        nc.vector.tensor_reduce(
                            out=fo[:, nsl].bitcast(F32R),
                            in_=h2[:gp2, :].rearrange("c (n k) -> c n k",
                                                      k=K),
                            axis=AX.X, op=OP.max)
                for gi in range(2):
                    fo = x3Ta if gi == 0 else x3Tb
                    ev = nc.gpsimd if V_GPSIMD_ELEM else nc.vector
                    ev.tensor_scalar(
                        fo[:, tsl].bitcast(F32R), fo[:, tsl], b3b[gi][:],
                        None, op0=OP.add)

            # ---------------- final MLP (per 512-node chunk) ----------------
            featc = [x1T[0:64, :], x2T, x3Ta, x3Tb]

            def mlp_chunk(j):
                fsl = slice(j * 512, (j + 1) * 512)
                h1c = [scr.tile([128, 512], F32, tag=f"mh1_{m}", bufs=1,
                                name=f"mh1_{m}") for m in range(4)]
                for m in range(4):
                    ps = pse.tile([128, 512], F32, tag="pre")
                    for ci, wc in enumerate(m1):
                        nc.tensor.matmul(
                            ps[:], _r(wc[:, m * 128:(m + 1) * 128]),
                            _r(featc[ci][:, fsl]),
                            start=(ci == 0), stop=(ci == 3))
                    nc.scalar.activation(h1c[m][:].bitcast(F32R), ps[:],
                                         ACTF.Relu, bias=bm1[m][:])
                h2c = [scr.tile([128, 512], F32, tag=f"mh2_{m}", bufs=1,
                                name=f"mh2_{m}") for m in range(4)]
                for m in range(4):
                    ps = pse.tile([128, 512], F32, tag="pre")
                    for ci in range(4):
                        nc.tensor.matmul(
                            ps[:], _r(m2[ci][:, m * 128:(m + 1) * 128]),
                            _r(h1c[ci][:]),
                            start=(ci == 0), stop=(ci == 3))
                    nc.scalar.activation(h2c[m][:].bitcast(F32R), ps[:],
                                         ACTF.Relu, bias=bm2[m][:])
                h3c = [scr.tile([128, 512], F32, tag=f"mh3_{m}", bufs=1,
                                name=f"mh3_{m}") for m in range(2)]
                for m in range(2):
                    ps = pse.tile([128, 512], F32, tag="pre")
                    for ci in range(4):
                        nc.tensor.matmul(
                            ps[:], _r(m3[ci][:, m * 128:(m + 1) * 128]),
                            _r(h2c[ci][:]),
                            start=(ci == 0), stop=(ci == 3))
                    nc.scalar.activation(h3c[m][:], ps[:], ACTF.Relu,
                                         bias=bm3[m][:])

                # classifier staged per chunk: all Exp first, then all Ln
                # (keeps the ACT function-table from thrashing per tile)
                hms, ssums = [], []
                for st in range(4):
                    tsl = slice(j * 512 + st * 128, j * 512 + st * 128 + 128)
                    lsl = slice(st * 128, (st + 1) * 128)
                    o_ps = psi.tile([128, 2], F32, tag="tmp")
                    nc.tensor.matmul(o_ps[:], h3c[0][:, lsl], m4[0][:],
                                     start=True, stop=False)
                    nc.tensor.matmul(o_ps[:], h3c[1][:, lsl], m4[1][:],
                                     start=False, stop=False)
                    nc.tensor.matmul(o_ps[:], ones1[:, tsl], bm4[:],
                                     start=False, stop=True)
                    mx = scr.tile([128, 1], F32, tag=f"mx_{st}")
                    nc.vector.tensor_reduce(out=mx[:], in_=o_ps[:], axis=AX.X,
                                            op=OP.max)
                    hm = scr.tile([128, 2], F32, tag=f"hm_{st}")
                    nc.vector.tensor_scalar(hm[:], o_ps[:], mx[:], None,
                                            op0=OP.subtract)
                    ex = scr.tile([128, 2], F32, tag=f"ex_{st}")
                    ssum = scr.tile([128, 1], F32, tag=f"ssum_{st}")
                    nc.scalar.activation(ex[:], hm[:], ACTF.Exp,
                                         accum_out=ssum[:])
                    hms.append(hm)
                    ssums.append(ssum)
                for st in range(4):
                    tsl = slice(j * 512 + st * 128, j * 512 + st * 128 + 128)
                    lns = scr.tile([128, 1], F32, tag=f"lns_{st}")
                    nc.scalar.activation(lns[:], ssums[st][:], ACTF.Ln)
                    res = scr.tile([128, 2], F32, tag=f"res_{st}")
                    nc.vector.tensor_scalar(res[:], hms[st][:], lns[:], None,
                                            op0=OP.subtract)
                    nc.sync.dma_start(out_d.ap()[tsl, :], res[:])

            # L3 groups produce x3T tiles 2g, 2g+1; MLP chunk j needs tiles
            # 4j..4j+3 (tail groups 2j, 2j+1) -> emit chunk j at g == 2j+2.
            prev = None
            for g in range(8):
                xg = l3_heads(g)
                if prev is not None:
                    pg, pxg = prev
                    for h in range(2):
                        l3_tail(2 * pg + h, h, pxg)
                if dbg_stop != 3 and g >= 2 and g % 2 == 0:
                    mlp_chunk((g - 2) // 2)
                prev = (g, xg)
            pg, pxg = prev
            for h in range(2):
                l3_tail(2 * pg + h, h, pxg)

            if dbg_stop == 3:
                nc.sync.dma_start(out_d.ap().rearrange("n c -> c n"),
                                  x3Ta[0:2, :])
                return

            mlp_chunk(2)
            mlp_chunk(3)


_NC_CACHE = {}


def _get_nc(repeat_loop=False):
    key = repeat_loop
    if key not in _NC_CACHE:
        _NC_CACHE[key] = build_nc(repeat_loop)
    return _NC_CACHE[key]


def kernel(**inputs):
    nc = _get_nc()
    in_maps = []
    for g in range(B):
        m = {"x": np.ascontiguousarray(np.asarray(inputs["x"][g], np.float32))}
        for w in WEIGHT_NAMES:
            m[w] = np.ascontiguousarray(np.asarray(inputs[w], np.float32))
        in_maps.append(m)
    res = bass_utils.run_bass_kernel_spmd(nc, in_maps, core_ids=list(range(B)))
    return np.stack([res.results[g]["out"] for g in range(B)], axis=0)

